# revision 7
# baseline (speedup 1.0000x reference)
"""FFTConv2d kernel for trn2, 8 NeuronCores.

Math: reference einsum 'bchw,oihw->bohw' factorizes:
  Y[b,o] = conv_full(sum_c x[b,c], sum_i w[o,i])[1:-1,1:-1] + bias[o]
i.e. a single-channel 3x3 "same" convolution (flipped kernel) per (b,o).

Per core (2 batches):
  1. DMA x slice in as bf16 hi/lo pair (exact fp32 split), partitions=(b,c).
  2. Channel-sum via PE matmul with ones-indicator lhsT -> PSUM [18, n]
     (9 replicated copies per batch), accumulating hi+lo passes.
  3. Copy PSUM -> padded staging SBUF [18, 34*130] (row stride 130, zero
     borders), rounding to fp32r.
  4. Build P [20, SH*130]: partition (b,k) = staging shifted by
     offk=(2-j)*130+(2-i); each shift is ONE contiguous SBUF->SBUF DMA.
     Partitions (b,9) hold ones (bias row).
  5. Conv = one fp32r matmul lhsT[20,128] @ P[20,390-chunk] -> PSUM [128,..]
     giving all (b,o) output images; bias via the ones row.
  6. Copy PSUM -> Y SBUF (dropping the 2 pad columns per 130-row),
     DMA Y -> HBM.
Processed in NS row-slices for DMA/compute overlap.
"""

import os
import sys
from functools import lru_cache

import numpy as np

for _p in ("/opt/trn_rl_repo", "/root/.axon_site/_ro/trn_rl_repo"):
    if os.path.isdir(_p) and _p not in sys.path:
        sys.path.insert(0, _p)

import ml_dtypes

B, CIN, COUT, H, W = 16, 64, 64, 128, 128
N_CORES = 8
BPC = B // N_CORES  # batches per core = 2
NS = 4  # row slices per core
SH = H // NS  # rows per slice = 32
WROW = W + 2  # padded row stride = 130
PWIN = SH * WROW  # P window per slice = 4160
MAXOFF = 2 * WROW + 2  # max shift offset = 262
SPLEN = PWIN + MAXOFF  # staging length = 4422
NPART = BPC * CIN  # 128 input partitions (b, c)
KP = 10  # P partitions per batch: 9 shifts + ones row
NOUT = BPC * COUT  # 128 output partitions (b, o)


@lru_cache(maxsize=1)
def _build():
    import concourse.bacc as bacc
    import concourse.mybir as mybir
    import concourse.tile as tile
    from concourse.ap import AP

    f32 = mybir.dt.float32
    f32r = mybir.dt.float32r
    bf16 = mybir.dt.bfloat16

    nc = bacc.Bacc("TRN2", target_bir_lowering=False, debug=False, num_devices=N_CORES)

    xhi = nc.dram_tensor("xhi", [NPART, H * W], bf16, kind="ExternalInput")
    xlo = nc.dram_tensor("xlo", [NPART, H * W], bf16, kind="ExternalInput")
    ones_cs = nc.dram_tensor("ones_cs", [NPART, BPC * 9], bf16, kind="ExternalInput")
    wb = nc.dram_tensor("wb", [BPC * KP, NOUT], f32r, kind="ExternalInput")
    ones_p = nc.dram_tensor("ones_p", [1, PWIN], f32r, kind="ExternalInput")
    y = nc.dram_tensor("y", [NOUT, H * W], f32, kind="ExternalOutput")

    RMAX = SH + 2

    with tile.TileContext(nc) as tc:
        with (
            tc.tile_pool(name="xin", bufs=3) as xin_pool,
            tc.tile_pool(name="sp", bufs=1) as sp_pool,
            tc.tile_pool(name="pbuf", bufs=1) as p_pool,
            tc.tile_pool(name="yout", bufs=3) as y_pool,
            tc.tile_pool(name="consts", bufs=1) as c_pool,
            tc.tile_pool(name="cs_ps", bufs=4, space="PSUM") as cs_psum,
            tc.tile_pool(name="cv_ps", bufs=4, space="PSUM") as cv_psum,
        ):
            ones_t = c_pool.tile([NPART, BPC * 9], bf16, tag="ones_cs")
            wb_t = c_pool.tile([BPC * KP, NOUT], f32r, tag="wb")
            nc.sync.dma_start(out=ones_t[:, :], in_=ones_cs.ap()[:, :])
            nc.sync.dma_start(out=wb_t[:, :], in_=wb.ap()[:, :])

            # rotating staging + P buffers (zeros in staging borders persist)
            spbufs = []
            pbufs = []
            for pi in range(min(3, NS)):
                sp = sp_pool.tile([BPC * 9, SPLEN], f32r, tag=f"SP{pi}")
                nc.gpsimd.memset(sp[:, :].bitcast(f32), 0.0)
                spbufs.append(sp)
                pb = p_pool.tile([BPC * KP, PWIN], f32r, tag=f"P{pi}")
                for b in range(BPC):
                    nc.sync.dma_start(
                        out=pb[b * KP + 9 : b * KP + 10, :],
                        in_=ones_p.ap()[0:1, :],
                    )
                pbufs.append(pb)

            for s in range(NS):
                hbase = SH * s - 1  # staging v-row 0 = image row hbase
                h0 = max(0, hbase)
                he = min(H, SH * s + SH + 1)
                R = he - h0
                ncols = R * W

                xhi_t = xin_pool.tile([NPART, RMAX * W], bf16, tag="xhi")
                xlo_t = xin_pool.tile([NPART, RMAX * W], bf16, tag="xlo")
                nc.sync.dma_start(
                    out=xhi_t[:, :ncols], in_=xhi.ap()[:, h0 * W : he * W]
                )
                nc.sync.dma_start(
                    out=xlo_t[:, :ncols], in_=xlo.ap()[:, h0 * W : he * W]
                )

                sp = spbufs[s % 3]
                spt = sp.tensor
                pb = pbufs[s % 3]

                if s == NS - 1:
                    # bottom border: zero staging rows beyond image row 127
                    vz = (H - hbase) * WROW
                    nc.vector.memset(sp[:, vz:SPLEN].bitcast(f32), 0.0)

                # channel sum: ones^T @ [xhi; xlo], PSUM -> padded staging
                nchunks = (ncols + 511) // 512
                for ci in range(nchunks):
                    c0 = ci * 512
                    cn = min(512, ncols - c0)
                    nrows = cn // W
                    ps = cs_psum.tile([BPC * 9, 4, W], f32, tag="cs")
                    nc.tensor.matmul(
                        ps[:, :nrows, :],
                        ones_t[:, :],
                        xhi_t[:, c0 : c0 + cn],
                        start=True,
                        stop=False,
                    )
                    nc.tensor.matmul(
                        ps[:, :nrows, :],
                        ones_t[:, :],
                        xlo_t[:, c0 : c0 + cn],
                        start=False,
                        stop=True,
                    )
                    v0 = (h0 + 4 * ci - hbase) * WROW + 1
                    dst = AP(
                        tensor=spt,
                        offset=v0,
                        ap=[[SPLEN, BPC * 9], [WROW, nrows], [1, W]],
                    )
                    src = ps[:, :nrows, :]
                    if ci % 2 == 0:
                        nc.vector.tensor_copy(dst, src)
                    else:
                        nc.scalar.copy(dst, src)

                # build P: per-(b,k) contiguous shifted copy from staging
                for b in range(BPC):
                    for k in range(9):
                        j, i = divmod(k, 3)
                        offk = (2 - j) * WROW + (2 - i)
                        eng = nc.gpsimd if k % 2 == 0 else nc.sync
                        eng.dma_start(
                            out=pb[b * KP + k : b * KP + k + 1, :],
                            in_=sp[b * 9 + k : b * 9 + k + 1, offk : offk + PWIN],
                        )

                # conv: wb^T @ P in chunks of 3 output rows
                yt = y_pool.tile([NOUT, SH, W], f32, tag="yout")
                nchunk = (SH + 2) // 3
                for c in range(nchunk):
                    rr0 = c * 3
                    nrr = min(3, SH - rr0)
                    nn = nrr * WROW
                    ps = cv_psum.tile([NOUT, 3, WROW], f32, tag="cv")
                    nc.tensor.matmul(
                        ps[:, :nrr, :],
                        wb_t[:, :],
                        pb[:, rr0 * WROW : rr0 * WROW + nn],
                        start=True,
                        stop=True,
                    )
                    if c % 2 == 0:
                        nc.vector.tensor_copy(
                            yt[:, rr0 : rr0 + nrr, :], ps[:, :nrr, 0:W]
                        )
                    else:
                        nc.scalar.copy(yt[:, rr0 : rr0 + nrr, :], ps[:, :nrr, 0:W])

                nc.gpsimd.dma_start(
                    out=y.ap()[:, SH * s * W : SH * (s + 1) * W],
                    in_=yt[:, :, :],
                )

    nc.compile()
    return nc


def _host_prep(x, weight, bias):
    bf = ml_dtypes.bfloat16
    wsum = weight.sum(axis=1)  # [COUT, 3, 3]
    wb = np.zeros((BPC * KP, NOUT), np.float32)
    for b in range(BPC):
        for j in range(3):
            for i in range(3):
                wb[b * KP + j * 3 + i, b * COUT : (b + 1) * COUT] = wsum[:, j, i]
        wb[b * KP + 9, b * COUT : (b + 1) * COUT] = bias
    ones_cs = np.zeros((NPART, BPC * 9), np.float32)
    for b in range(BPC):
        ones_cs[b * CIN : (b + 1) * CIN, b * 9 : (b + 1) * 9] = 1.0
    ones_cs = ones_cs.astype(bf)
    ones_p = np.ones((1, PWIN), np.float32)

    in_maps = []
    for r in range(N_CORES):
        xs = np.ascontiguousarray(
            x[r * BPC : (r + 1) * BPC].reshape(NPART, H * W)
        ).astype(np.float32)
        xhi = xs.astype(bf)
        xlo = (xs - xhi.astype(np.float32)).astype(bf)
        in_maps.append(
            {
                "xhi": xhi,
                "xlo": xlo,
                "ones_cs": ones_cs,
                "wb": wb,
                "ones_p": ones_p,
            }
        )
    return in_maps


def kernel(x, weight, bias):
    from concourse.bass_utils import run_bass_kernel_spmd

    x = np.asarray(x)
    weight = np.asarray(weight)
    bias = np.asarray(bias)
    nc = _build()
    in_maps = _host_prep(x, weight, bias)
    res = run_bass_kernel_spmd(nc, in_maps, core_ids=list(range(N_CORES)))
    out = np.concatenate(
        [
            res.results[r]["y"].reshape(BPC, COUT, H, W)
            for r in range(N_CORES)
        ],
        axis=0,
    )
    return out.astype(np.float32)
